# revision 11
# baseline (speedup 1.0000x reference)
"""Trainium2 Bass kernel for the CSMHP (clustered self-exciting Hawkes process)
negative log-likelihood, distributed over 8 NeuronCores.

Math
----
The reference builds the full (C, N, N) pairwise decay tensor and row-reduces
it with logsumexp.  The excitation

    E[c, i] = sum_{j<i} exp(-beta_c * (t_i - t_j))

obeys the first-order recurrence  E_i = d_i * (E_{i-1} + 1)  with
d_i = exp(-beta_c * (t_i - t_{i-1})), which maps exactly onto the DVE
`tensor_tensor_scan` instruction: state = (d *mult* state) *add* d.
That turns the O(N^2 C) pairwise tensor into O(N C) work.

Sharding
--------
Events are split into 8 contiguous blocks of 512 (the N axis of the pairwise
tensor, as the hint suggests).  Each core:
  * computes its scan-initial state A'[c] = E[c, first_own_event - 1] directly
    from the (padded, uniform-shape) list of prior events — a dense
    exp+reduce over at most 3584 values, so no cross-core recurrence and no
    collectives are needed;
  * runs the scan over its 512 events for all 8 clusters at once;
  * reduces its partial log-likelihood sum, its partial probability column
    sum, and (core 7) the excitation at the very last event, which is exactly
    the data the analytic integral term needs.
The host sums the 8 partial scalars (the "all-reduce" of the hint).
"""

import numpy as np

import concourse.bass as bass
import concourse.tile as tile
from concourse import mybir
from concourse.bass_utils import run_bass_kernel_spmd
from concourse.vector_clock import ScopedClock

F32 = mybir.dt.float32
ALU = mybir.AluOpType
ACT = mybir.ActivationFunctionType

N = 4096
C = 8
NCORES = 8
CHUNK = N // NCORES          # 512 events per core
PRIOR_PAD = 3584             # max prior events (core 7: 512*7-1=3583), padded
PCOLS = PRIOR_PAD // 128     # 28
T_WINDOW = 100.0
BIG = 1.0e9                  # pad offset: exp(-beta*BIG) == 0 in fp32

_NC_CACHE = None


class SplitDrainTileContext(tile.TileContext):
    """TileContext whose kernel-tail drain carries one sem wait per Drain.

    The TPB instruction encoding in this toolchain holds a single embedded
    semaphore wait; the stock TileContext attaches every outstanding proc
    semaphore to one Drain, which walrus rejects with "Too many sync wait
    commands".  Emitting a chain of Drains (one wait each) on the same SP
    queue is sequentially equivalent.
    """

    def _drain_and_barrier(self, tick_clock, wait_clock):
        drain_inst = self.nc.sync.drain()
        wait_clock.add_sem_waits(
            drain_inst.ins, ScopedClock({None: tick_clock.global_clock})
        )
        si = drain_inst.ins.sync_info
        if si is not None and si.on_wait and len(si.on_wait) > 1:
            waits = list(si.on_wait)
            drain_inst.ins.sync_info = mybir.SyncInfo(
                on_wait=[waits[0]], on_update=list(si.on_update or [])
            )
            for w in waits[1:]:
                extra = self.nc.sync.drain()
                extra.ins.sync_info = mybir.SyncInfo(on_wait=[w], on_update=[])

        self.nc.all_engine_barrier()
        assert self.sems is not None
        popped = self.nc._tile_sem_poison_stack.pop()
        assert popped is self._sem_poison
        self.nc.clear_and_free_semaphores(list(self.sems.allocated().values()))
        self.nc.all_engine_barrier()


def _build_nc():
    """Build the (SPMD-uniform) Bass program run on every core.

    Sync-wait budget: the CoreV3 DVE instruction encoding holds a single
    embedded semaphore wait, so every DVE op may depend on at most ONE
    foreign processor.  All 8-partition inputs travel in one DMA (inA), all
    128-partition inputs in another (inB), and the PSUM matmul result is
    bounced through an ACT copy so the scan's producers are both ACT.
    """
    nc = bass.Bass("TRN2", target_bir_lowering=False, debug=False)

    # inA columns: [0:512] t_own | [512:1024] t_prev | [1024:1536] pT
    #              [1536:1540] scal (beta, alpha, mu, gamma)
    ina_d = nc.dram_tensor("inA", [C, 3 * CHUNK + 4], F32, kind="ExternalInput")
    # inB columns: [0:28] prior | [28:29] tref | [29:37] B128
    inb_d = nc.dram_tensor("inB", [128, PCOLS + 1 + C], F32, kind="ExternalInput")

    ll_out_d = nc.dram_tensor("ll_out", [1, 1], F32, kind="ExternalOutput")
    psum_out_d = nc.dram_tensor("psum_out", [C, 1], F32, kind="ExternalOutput")
    elast_out_d = nc.dram_tensor("elast_out", [C, 1], F32, kind="ExternalOutput")

    with SplitDrainTileContext(nc) as tc:
        with (
            tc.tile_pool(name="sb", bufs=1) as sb,
            tc.tile_pool(name="ps", bufs=1, space="PSUM") as ps,
        ):
            ina = sb.tile([C, 3 * CHUNK + 4], F32)
            nc.gpsimd.dma_start(out=ina, in_=ina_d.ap())
            inb = sb.tile([128, PCOLS + 1 + C], F32)
            nc.gpsimd.dma_start(out=inb, in_=inb_d.ap())

            t_own = ina[:, 0:CHUNK]
            t_prev = ina[:, CHUNK : 2 * CHUNK]
            pt = ina[:, 2 * CHUNK : 3 * CHUNK]
            scal = ina[:, 3 * CHUNK : 3 * CHUNK + 4]
            prior = inb[:, 0:PCOLS]
            tref = inb[:, PCOLS : PCOLS + 1]
            b128 = inb[:, PCOLS + 1 : PCOLS + 1 + C]

            beta_col = scal[:, 0:1]
            alpha_col = scal[:, 1:2]
            mu_col = scal[:, 2:3]
            gamma_col = scal[:, 3:4]

            # ones vector built on ACT from the freshly-DMA'd inB tile: doubles
            # as ACT's observation of the inB semaphore, so later ACT ops that
            # read inB slices plus a DVE tile need only the DVE wait.
            ones128 = sb.tile([128, 1], F32)
            nc.scalar.activation(
                ones128, tref, ACT.Identity, bias=1.0, scale=0.0
            )

            # ---- prolog: A'[c] = sum_j exp(beta_c * (prior_j - t_ref)) ----
            dpri = sb.tile([128, PCOLS], F32)
            nc.vector.tensor_scalar(
                out=dpri, in0=prior, scalar1=tref, scalar2=None, op0=ALU.subtract
            )
            r_part = sb.tile([128, C], F32)
            esc = sb.tile([128, PCOLS], F32)
            for c in range(C):
                nc.scalar.activation(
                    out=esc,
                    in_=dpri,
                    func=ACT.Exp,
                    scale=b128[:, c : c + 1],
                    accum_out=r_part[:, c : c + 1],
                )
            a_init = ps.tile([C, 1], F32)
            nc.tensor.matmul(a_init, r_part, ones128, start=True, stop=True)
            # bounce PSUM->SBUF on ACT so the scan's two producers share one
            # semaphore (DVE instructions hold a single embedded wait)
            a_init_sb = sb.tile([C, 1], F32)
            nc.scalar.copy(a_init_sb, a_init)

            # ---- decay factors and the excitation scan ----
            dt = sb.tile([C, CHUNK], F32)
            nc.vector.tensor_sub(dt, t_own, t_prev)
            negb = sb.tile([C, 1], F32)
            nc.vector.tensor_scalar_mul(negb, beta_col, -1.0)
            dec = sb.tile([C, CHUNK], F32)
            nc.scalar.activation(dec, dt, ACT.Exp, scale=negb)
            exc = sb.tile([C, CHUNK], F32)
            nc.vector.tensor_tensor_scan(
                exc, dec, dec, initial=a_init_sb, op0=ALU.mult, op1=ALU.add
            )

            # ---- intensities: lamb = alpha*E + mu + gamma*t/T ----
            lamb = sb.tile([C, CHUNK], F32)
            nc.vector.tensor_scalar(
                out=lamb, in0=exc, scalar1=alpha_col, scalar2=mu_col,
                op0=ALU.mult, op1=ALU.add,
            )
            gt = sb.tile([C, CHUNK], F32)
            nc.vector.tensor_scalar_mul(gt, t_own, gamma_col)
            lamb2 = sb.tile([C, CHUNK], F32)
            nc.vector.scalar_tensor_tensor(
                out=lamb2, in0=gt, scalar=1.0 / T_WINDOW, in1=lamb,
                op0=ALU.mult, op1=ALU.add,
            )
            pl = sb.tile([C, CHUNK], F32)
            nc.vector.tensor_mul(pl, lamb2, pt)

            # intensity[i] = sum_c pl[c, i]  (contract partitions via matmul)
            inten = ps.tile([1, CHUNK], F32)
            nc.tensor.matmul(inten, ones128[0:C, :], pl, start=True, stop=True)

            logi = sb.tile([1, CHUNK], F32)
            nc.scalar.activation(logi, inten, ACT.Ln)
            ll = sb.tile([1, 1], F32)
            nc.vector.reduce_sum(ll, logi, axis=mybir.AxisListType.X)
            ppart = sb.tile([C, 1], F32)
            nc.vector.reduce_sum(ppart, pt, axis=mybir.AxisListType.X)

            nc.gpsimd.dma_start(out=ll_out_d.ap(), in_=ll)
            nc.gpsimd.dma_start(out=psum_out_d.ap(), in_=ppart)
            nc.gpsimd.dma_start(out=elast_out_d.ap(), in_=exc[:, CHUNK - 1 : CHUNK])

    return nc


def get_nc():
    global _NC_CACHE
    if _NC_CACHE is None:
        _NC_CACHE = _build_nc()
    return _NC_CACHE


def make_in_maps(probability, event_times, mu, gamma, alpha_kernel, beta_kernel):
    t = np.ascontiguousarray(np.asarray(event_times, dtype=np.float32))
    p = np.ascontiguousarray(np.asarray(probability, dtype=np.float32))
    beta = np.asarray(beta_kernel, dtype=np.float32)
    alpha = np.asarray(alpha_kernel, dtype=np.float32)
    mu_ = np.asarray(mu, dtype=np.float32)
    gamma_ = np.asarray(gamma, dtype=np.float32)

    scal = np.stack([beta, alpha, mu_, gamma_], axis=1)
    b128 = np.broadcast_to(beta, (128, C))

    in_maps = []
    for k in range(NCORES):
        s = k * CHUNK
        t_own = np.broadcast_to(t[s : s + CHUNK], (C, CHUNK))
        tp = np.empty(CHUNK, np.float32)
        if k == 0:
            tp[0] = t[0] - BIG  # forces d_0 = 0: no events precede event 0
            tp[1:] = t[: CHUNK - 1]
        else:
            tp[:] = t[s - 1 : s + CHUNK - 1]
        t_prev = np.broadcast_to(tp, (C, CHUNK))
        pt = p[s : s + CHUNK, :].T

        npri = max(s - 1, 0)
        pri = np.full(PRIOR_PAD, -BIG, np.float32)
        pri[:npri] = t[:npri]
        prior_pm = pri.reshape(PCOLS, 128).T
        tref_val = t[s - 1] if k > 0 else t[0]
        tref = np.full((128, 1), tref_val, np.float32)

        ina = np.ascontiguousarray(
            np.concatenate([t_own, t_prev, pt, scal], axis=1, dtype=np.float32)
        )
        inb = np.ascontiguousarray(
            np.concatenate([prior_pm, tref, b128], axis=1, dtype=np.float32)
        )
        in_maps.append({"inA": ina, "inB": inb})
    return in_maps


def combine_outputs(results, event_times, mu, gamma, alpha_kernel, beta_kernel):
    """Host-side reduction of the per-core partial scalars (float64)."""
    t = np.asarray(event_times, dtype=np.float32)
    beta = np.asarray(beta_kernel, dtype=np.float64)
    alpha = np.asarray(alpha_kernel, dtype=np.float64)
    mu_ = np.asarray(mu, dtype=np.float64)
    gamma_ = np.asarray(gamma, dtype=np.float64)

    ll_sum = sum(float(r["ll_out"][0, 0]) for r in results)
    psum = np.zeros(C, np.float64)
    for r in results:
        psum += r["psum_out"][:, 0].astype(np.float64)
    elast = results[NCORES - 1]["elast_out"][:, 0].astype(np.float64)

    ab = alpha / beta
    exp_term = ab * ((N - 1) - elast)
    t_diff = float(t[-1]) - float(t[0])
    t_sq_diff = float(t[-1]) ** 2 - float(t[0]) ** 2
    base_terms = t_diff * mu_ + t_sq_diff * gamma_ / (2.0 * T_WINDOW)
    integral_part = float(psum @ (exp_term + base_terms)) / N
    return np.float32(-(ll_sum - integral_part))


def kernel(probability, event_times, mu, gamma, alpha_kernel, beta_kernel):
    nc = get_nc()
    in_maps = make_in_maps(
        probability, event_times, mu, gamma, alpha_kernel, beta_kernel
    )
    res = run_bass_kernel_spmd(nc, in_maps, core_ids=list(range(NCORES))).results
    return combine_outputs(
        res, event_times, mu, gamma, alpha_kernel, beta_kernel
    )


# revision 15
# speedup vs baseline: 1.2671x; 1.2671x over previous
"""Trainium2 Bass kernel for the CSMHP (clustered self-exciting Hawkes process)
negative log-likelihood, distributed over 8 NeuronCores.

Math
----
The reference builds the full (C, N, N) pairwise decay tensor and row-reduces
it with logsumexp.  The excitation

    E[c, i] = sum_{j<i} exp(-beta_c * (t_i - t_j))

obeys the first-order recurrence  E_i = d_i * (E_{i-1} + 1)  with
d_i = exp(-beta_c * (t_i - t_{i-1})), which maps exactly onto the DVE
`tensor_tensor_scan` instruction: state = (d *mult* state) *add* d.
That turns the O(N^2 C) pairwise tensor into O(N C) work.

Sharding
--------
Events are split into 8 contiguous blocks of 512 (the N axis of the pairwise
tensor, as the hint suggests).  Each core:
  * computes its scan-initial state A'[c] = E[c, first_own_event - 1] directly
    from the (padded, uniform-shape) list of prior events — a dense
    exp+reduce over at most 3584 values, so no cross-core recurrence and no
    collectives are needed;
  * runs the scan over its 512 events for all 8 clusters at once;
  * reduces its partial log-likelihood sum, its partial probability column
    sum, and (core 7) the excitation at the very last event, which is exactly
    the data the analytic integral term needs.
The host sums the 8 partial scalars (the "all-reduce" of the hint).
"""

import numpy as np

import concourse.bass as bass
import concourse.tile as tile
from concourse import mybir
from concourse.bass_utils import run_bass_kernel_spmd
from concourse.vector_clock import ScopedClock

F32 = mybir.dt.float32
ALU = mybir.AluOpType
ACT = mybir.ActivationFunctionType

N = 4096
C = 8
NCORES = 8
CHUNK = N // NCORES          # 512 events per core
PRIOR_PAD = 3584             # max prior events (core 7: 512*7-1=3583), padded
PCOLS = PRIOR_PAD // 128     # 28
T_WINDOW = 100.0
BIG = 1.0e9                  # pad offset: exp(-beta*BIG) == 0 in fp32

_NC_CACHE = None


class SplitDrainTileContext(tile.TileContext):
    """TileContext whose kernel-tail drain carries one sem wait per Drain.

    The TPB instruction encoding in this toolchain holds a single embedded
    semaphore wait; the stock TileContext attaches every outstanding proc
    semaphore to one Drain, which walrus rejects with "Too many sync wait
    commands".  Emitting a chain of Drains (one wait each) on the same SP
    queue is sequentially equivalent.
    """

    def _drain_and_barrier(self, tick_clock, wait_clock):
        drain_inst = self.nc.sync.drain()
        wait_clock.add_sem_waits(
            drain_inst.ins, ScopedClock({None: tick_clock.global_clock})
        )
        si = drain_inst.ins.sync_info
        if si is not None and si.on_wait and len(si.on_wait) > 1:
            waits = list(si.on_wait)
            drain_inst.ins.sync_info = mybir.SyncInfo(
                on_wait=[waits[0]], on_update=list(si.on_update or [])
            )
            for w in waits[1:]:
                extra = self.nc.sync.drain()
                extra.ins.sync_info = mybir.SyncInfo(on_wait=[w], on_update=[])

        self.nc.all_engine_barrier()
        assert self.sems is not None
        popped = self.nc._tile_sem_poison_stack.pop()
        assert popped is self._sem_poison
        self.nc.clear_and_free_semaphores(list(self.sems.allocated().values()))
        self.nc.all_engine_barrier()


def _build_nc():
    """Build the (SPMD-uniform) Bass program run on every core.

    Sync-wait budget: the CoreV3 DVE instruction encoding holds a single
    embedded semaphore wait, so every DVE op may depend on at most ONE
    foreign processor.  All 8-partition inputs travel in one DMA (inA), all
    128-partition inputs in another (inB), and the PSUM matmul result is
    bounced through an ACT copy so the scan's producers are both ACT.
    """
    nc = bass.Bass("TRN2", target_bir_lowering=False, debug=False)

    # inA columns: [0:512] t_own | [512:1024] t_prev | [1024:1536] pT
    #              [1536:1540] scal (beta, alpha, mu, gamma)
    ina_d = nc.dram_tensor("inA", [C, 3 * CHUNK + 4], F32, kind="ExternalInput")
    # inB columns: [0:28] prior | [28:29] tref | [29:37] B128 | [37:38] ones
    inb_d = nc.dram_tensor("inB", [128, PCOLS + 2 + C], F32, kind="ExternalInput")

    ll_out_d = nc.dram_tensor("ll_out", [1, 1], F32, kind="ExternalOutput")
    psum_out_d = nc.dram_tensor("psum_out", [C, 1], F32, kind="ExternalOutput")
    elast_out_d = nc.dram_tensor("elast_out", [C, 1], F32, kind="ExternalOutput")

    with SplitDrainTileContext(nc) as tc:
        with (
            tc.tile_pool(name="sb", bufs=1) as sb,
            tc.tile_pool(name="ps", bufs=1, space="PSUM") as ps,
        ):
            ina = sb.tile([C, 3 * CHUNK + 4], F32)
            nc.gpsimd.dma_start(out=ina, in_=ina_d.ap())
            inb = sb.tile([128, PCOLS + 2 + C], F32)
            nc.gpsimd.dma_start(out=inb, in_=inb_d.ap())

            t_own = ina[:, 0:CHUNK]
            t_prev = ina[:, CHUNK : 2 * CHUNK]
            pt = ina[:, 2 * CHUNK : 3 * CHUNK]
            scal = ina[:, 3 * CHUNK : 3 * CHUNK + 4]
            prior = inb[:, 0:PCOLS]
            tref = inb[:, PCOLS : PCOLS + 1]
            b128 = inb[:, PCOLS + 1 : PCOLS + 1 + C]
            ones_in = inb[:, PCOLS + 1 + C : PCOLS + 2 + C]

            beta_col = scal[:, 0:1]
            alpha_col = scal[:, 1:2]
            mu_col = scal[:, 2:3]
            gamma_col = scal[:, 3:4]

            # ---- prolog: A'[c] = sum_j exp(beta_c * (prior_j - t_ref)) ----
            # wbig[p, c, j] = (prior[p, j] - tref[p]) * beta_c, one DVE op via
            # zero-stride broadcast APs
            prior_b = bass.AP(
                tensor=prior.tensor, offset=prior.offset,
                ap=[prior.ap[0], [0, C], prior.ap[1]],
            )
            b128_b = bass.AP(
                tensor=b128.tensor, offset=b128.offset,
                ap=[b128.ap[0], b128.ap[1], [0, PCOLS]],
            )
            wbig = sb.tile([128, C, PCOLS], F32)
            nc.vector.scalar_tensor_tensor(
                out=wbig, in0=prior_b, scalar=tref, in1=b128_b,
                op0=ALU.subtract, op1=ALU.mult,
            )
            ebig = sb.tile([128, C, PCOLS], F32)
            nc.scalar.activation(ebig, wbig, ACT.Exp)
            r_part = sb.tile([128, C], F32)
            nc.vector.reduce_sum(r_part, ebig, axis=mybir.AxisListType.X)

            # ones column bounced through DVE so the matmul's two producers
            # (r_part, ones_col) share the DVE semaphore: one embedded wait
            ones_col = sb.tile([128, 1], F32)
            nc.vector.tensor_copy(ones_col, ones_in)
            a_init = ps.tile([C, 1], F32)
            nc.tensor.matmul(a_init, r_part, ones_col, start=True, stop=True)
            # PSUM->SBUF bounce on ACT: the scan's two producers (dec and
            # a_init_sb) then share the single ACT semaphore.  A DVE bounce
            # would cost the scan a same-engine RAW wait on top of the ACT
            # wait (DVE's pipeline is not interlocked), exceeding the one
            # embedded wait the encoding allows.
            a_init_sb = sb.tile([C, 1], F32)
            nc.scalar.copy(a_init_sb, a_init)

            # ---- decay factors and the excitation scan ----
            dt = sb.tile([C, CHUNK], F32)
            nc.vector.tensor_sub(dt, t_own, t_prev)
            negb = sb.tile([C, 1], F32)
            nc.vector.tensor_scalar_mul(negb, beta_col, -1.0)
            dec = sb.tile([C, CHUNK], F32)
            nc.scalar.activation(dec, dt, ACT.Exp, scale=negb)
            exc = sb.tile([C, CHUNK], F32)
            nc.vector.tensor_tensor_scan(
                exc, dec, dec, initial=a_init_sb, op0=ALU.mult, op1=ALU.add
            )

            # ---- intensities: lamb = alpha*E + mu + gamma*t/T ----
            lamb = sb.tile([C, CHUNK], F32)
            nc.vector.tensor_scalar(
                out=lamb, in0=exc, scalar1=alpha_col, scalar2=mu_col,
                op0=ALU.mult, op1=ALU.add,
            )
            gt = sb.tile([C, CHUNK], F32)
            nc.vector.tensor_scalar_mul(gt, t_own, gamma_col)
            lamb2 = sb.tile([C, CHUNK], F32)
            nc.vector.scalar_tensor_tensor(
                out=lamb2, in0=gt, scalar=1.0 / T_WINDOW, in1=lamb,
                op0=ALU.mult, op1=ALU.add,
            )
            pl = sb.tile([C, CHUNK], F32)
            nc.vector.tensor_mul(pl, lamb2, pt)

            # intensity[i] = sum_c pl[c, i]  (contract partitions via matmul)
            inten = ps.tile([1, CHUNK], F32)
            nc.tensor.matmul(inten, ones_col[0:C, :], pl, start=True, stop=True)

            # Ln + free-dim sum fused: accum_out gives sum_i log(intensity_i)
            logi = sb.tile([1, CHUNK], F32)
            ll = sb.tile([1, 1], F32)
            nc.scalar.activation(logi, inten, ACT.Ln, accum_out=ll)
            ppart = sb.tile([C, 1], F32)
            nc.vector.reduce_sum(ppart, pt, axis=mybir.AxisListType.X)

            nc.gpsimd.dma_start(out=ll_out_d.ap(), in_=ll)
            nc.gpsimd.dma_start(out=psum_out_d.ap(), in_=ppart)
            nc.gpsimd.dma_start(out=elast_out_d.ap(), in_=exc[:, CHUNK - 1 : CHUNK])

    return nc


def get_nc():
    global _NC_CACHE
    if _NC_CACHE is None:
        _NC_CACHE = _build_nc()
    return _NC_CACHE


def make_in_maps(probability, event_times, mu, gamma, alpha_kernel, beta_kernel):
    t = np.ascontiguousarray(np.asarray(event_times, dtype=np.float32))
    p = np.ascontiguousarray(np.asarray(probability, dtype=np.float32))
    beta = np.asarray(beta_kernel, dtype=np.float32)
    alpha = np.asarray(alpha_kernel, dtype=np.float32)
    mu_ = np.asarray(mu, dtype=np.float32)
    gamma_ = np.asarray(gamma, dtype=np.float32)

    scal = np.stack([beta, alpha, mu_, gamma_], axis=1)
    b128 = np.broadcast_to(beta, (128, C))

    in_maps = []
    for k in range(NCORES):
        s = k * CHUNK
        t_own = np.broadcast_to(t[s : s + CHUNK], (C, CHUNK))
        tp = np.empty(CHUNK, np.float32)
        if k == 0:
            tp[0] = t[0] - BIG  # forces d_0 = 0: no events precede event 0
            tp[1:] = t[: CHUNK - 1]
        else:
            tp[:] = t[s - 1 : s + CHUNK - 1]
        t_prev = np.broadcast_to(tp, (C, CHUNK))
        pt = p[s : s + CHUNK, :].T

        npri = max(s - 1, 0)
        pri = np.full(PRIOR_PAD, -BIG, np.float32)
        pri[:npri] = t[:npri]
        prior_pm = pri.reshape(PCOLS, 128).T
        tref_val = t[s - 1] if k > 0 else t[0]
        tref = np.full((128, 1), tref_val, np.float32)

        ina = np.ascontiguousarray(
            np.concatenate([t_own, t_prev, pt, scal], axis=1, dtype=np.float32)
        )
        ones_c = np.ones((128, 1), np.float32)
        inb = np.ascontiguousarray(
            np.concatenate([prior_pm, tref, b128, ones_c], axis=1, dtype=np.float32)
        )
        in_maps.append({"inA": ina, "inB": inb})
    return in_maps


def combine_outputs(results, event_times, mu, gamma, alpha_kernel, beta_kernel):
    """Host-side reduction of the per-core partial scalars (float64)."""
    t = np.asarray(event_times, dtype=np.float32)
    beta = np.asarray(beta_kernel, dtype=np.float64)
    alpha = np.asarray(alpha_kernel, dtype=np.float64)
    mu_ = np.asarray(mu, dtype=np.float64)
    gamma_ = np.asarray(gamma, dtype=np.float64)

    ll_sum = sum(float(r["ll_out"][0, 0]) for r in results)
    psum = np.zeros(C, np.float64)
    for r in results:
        psum += r["psum_out"][:, 0].astype(np.float64)
    elast = results[NCORES - 1]["elast_out"][:, 0].astype(np.float64)

    ab = alpha / beta
    exp_term = ab * ((N - 1) - elast)
    t_diff = float(t[-1]) - float(t[0])
    t_sq_diff = float(t[-1]) ** 2 - float(t[0]) ** 2
    base_terms = t_diff * mu_ + t_sq_diff * gamma_ / (2.0 * T_WINDOW)
    integral_part = float(psum @ (exp_term + base_terms)) / N
    return np.float32(-(ll_sum - integral_part))


def kernel(probability, event_times, mu, gamma, alpha_kernel, beta_kernel):
    nc = get_nc()
    in_maps = make_in_maps(
        probability, event_times, mu, gamma, alpha_kernel, beta_kernel
    )
    res = run_bass_kernel_spmd(nc, in_maps, core_ids=list(range(NCORES))).results
    return combine_outputs(
        res, event_times, mu, gamma, alpha_kernel, beta_kernel
    )
